# revision 21
# baseline (speedup 1.0000x reference)
"""Min-Euclidean-distance retrieval kernel for Trainium2 (8 NeuronCores).

Reference computation:
    x: [1, 2048, 512], y: [1, 65536, 512] (fp32)
    sq[p, r] = ||x_p||^2 + ||y_r||^2 - 2 <x_p, y_r>
    out = min over (p, r) of sqrt(max(sq, 0))

Sharding: the candidate pool (R) is split across 8 cores, 8192 candidates
each. The host pre-arranges both GEMM operands partition-major in bf16 so
each DMA moves long contiguous per-partition runs and the contraction dim
(d) lands on SBUF partitions with no on-chip transposes.

Per core, per PSUM tile [128 candidates x 512 queries] (bf16 matmuls run
the PE at 1 cycle/row):
  ScalarE:  h = -2*G + y2[r]          (per-partition bias, fp32)
  VectorE:  acc = min(acc, h)         (elementwise across candidate tiles)
The per-query ||x_p||^2 term is constant across candidates, so it is added
on the host, along with the final min across lanes/cores and the
(monotone) sqrt. bf16 operands cost ~1e-4 relative error on the final
distance, far inside tolerance; y2/x2 stay fp32.
"""

import sys

for _p in ("/opt/trn_rl_repo", "/root/.axon_site/_ro/trn_rl_repo"):
    if _p not in sys.path:
        sys.path.append(_p)

import ml_dtypes
import numpy as np

import concourse.bass as bass
import concourse.mybir as mybir
import concourse.tile as tile
from concourse import bacc, bass_utils

P = 2048          # queries
R = 65536         # candidates (full)
D = 512           # feature dim
NCORES = 8
R_LOC = R // NCORES      # 8192 candidates per core
P_CHUNKS = P // 512      # 4 moving chunks of queries
R_TILES = R_LOC // 128   # 64 stationary tiles of candidates
R_GROUPS = 16            # DMA granularity for y: 512 candidates per group
K_TILES = D // 128       # 4 contraction tiles

F32 = mybir.dt.float32
BF16 = mybir.dt.bfloat16
# fp8 e4m3 + DoubleRow runs the PE at 2x bf16 rate; measured rel err on the
# reference inputs is ~1.6e-3 vs 1.1e-4 for bf16. Set USE_FP8=False
# to fall back to the safe bf16 kernel (243 us, 1.1e-4).
USE_FP8 = True
# In fp8 mode the epilogue (ACT combine + DVE min chain) runs in bf16 for
# 2x engine modes. A constant shift keeps the values that matter (near the
# global min, sq ~ 650 => h ~ 150) in a bf16 range with quantum ~1, so the
# extra quantization error (~2e-4 on the distance) is negligible next to
# the fp8 GEMM noise. min-accumulation itself is exact in any format.
Y2_SHIFT = np.float32(512.0)
if USE_FP8:
    MM_DT = mybir.dt.float8e4
    MM_NP = ml_dtypes.float8_e4m3
    ACC_DT, ACC_NP = BF16, ml_dtypes.bfloat16
else:
    MM_DT = mybir.dt.bfloat16
    MM_NP = ml_dtypes.bfloat16
    ACC_DT, ACC_NP = F32, np.float32


def _build_module() -> bass.Bass:
    nc = bacc.Bacc("TRN2", target_bir_lowering=False, debug=False)

    # Host-prepared layouts (partition-major, contiguous per partition):
    #   xt[q, c, k, j] = x[c*512 + j, k*128 + q]
    #   yt[q, g, k, s] = y[g*512 + s, k*128 + q]
    #   y2t[lane, t]   = ||y_r||^2 for r = t*128 + lane
    xt = nc.dram_tensor("xt", [128, P_CHUNKS, K_TILES, 512], MM_DT,
                        kind="ExternalInput")
    yt = nc.dram_tensor("yt", [128, R_GROUPS, K_TILES, 512], MM_DT,
                        kind="ExternalInput")
    y2t = nc.dram_tensor("y2t", [128, R_TILES], F32, kind="ExternalInput")
    # acc[lane, c*512 + j] = min over r-tiles t of
    #   (y2[t*128+lane] - 2 G[t*128+lane, c*512+j])
    out = nc.dram_tensor("out", [128, P], ACC_DT, kind="ExternalOutput")

    with tile.TileContext(nc) as tc:
        with (
            tc.tile_pool(name="big", bufs=1) as big,
            tc.tile_pool(name="scr", bufs=4) as scr,
            tc.tile_pool(name="psum", bufs=(2 if USE_FP8 else 8), space="PSUM") as psum,
        ):
            xt_sb = big.tile([128, P_CHUNKS, K_TILES, 512], MM_DT)
            yt_sb = big.tile([128, R_GROUPS, K_TILES, 512], MM_DT)
            y2t_sb = big.tile([128, R_TILES], F32)
            acc = big.tile([128, P], ACC_DT)

            # First PSUM tile needs x chunk 0 + y group 0 (+ y2t before the
            # first ACT). x goes on the scalar HWDGE ring, y on the sync
            # ring (parallel), y2t via gpsimd SWDGE (its 256B/partition
            # descriptors crawl on the HWDGE ring). Per-group y DMAs let
            # matmuls unblock progressively instead of waiting for one
            # monolithic transfer.
            nc.scalar.dma_start(xt_sb[:, 0], xt.ap()[:, 0])
            nc.sync.dma_start(yt_sb[:, 0], yt.ap()[:, 0])
            nc.gpsimd.dma_start(y2t_sb[:], y2t.ap())
            for g in range(1, R_GROUPS):
                nc.sync.dma_start(yt_sb[:, g], yt.ap()[:, g])
            for c in range(1, P_CHUNKS):
                nc.scalar.dma_start(xt_sb[:, c], xt.ap()[:, c])

            if USE_FP8:
                # DoubleRow: each matmul contracts 2x128 packed K-rows at
                # 2 rows/cycle. One [128, 2048] PSUM tensor (4 banks) holds
                # all query chunks of a candidate tile, so the epilogue is a
                # single wide ACT + DVE op per tile (amortized overheads —
                # the serial DVE min chain is the throughput limit here).
                for t in range(R_TILES):
                    g, o = t // 4, (t % 4) * 128
                    pt = psum.tile([128, P], F32, name="pt")
                    for c in range(P_CHUNKS):
                        for kk in range(K_TILES // 2):
                            nc.tensor.matmul(
                                pt[:, c * 512 : (c + 1) * 512],
                                lhsT=yt_sb[:, g, 2 * kk : 2 * kk + 2, o : o + 128],
                                rhs=xt_sb[:, c, 2 * kk : 2 * kk + 2, :],
                                start=(kk == 0),
                                stop=(kk == K_TILES // 2 - 1),
                                perf_mode=mybir.MatmulPerfMode.DoubleRow,
                            )
                    bias = y2t_sb[:, t : t + 1]
                    if t == 0:
                        nc.scalar.activation(
                            out=acc[:],
                            in_=pt[:],
                            func=mybir.ActivationFunctionType.Identity,
                            bias=bias,
                            scale=-2.0,
                        )
                    else:
                        # Split the PSUM combine so ACT and DVE finish at
                        # the same time as the PE's 8 matmuls: ACT takes
                        # [0:SPLIT] (PSUM source pins it at 1x), DVE takes
                        # the rest via tensor_scalar, then does the bf16
                        # 2x-mode min for the whole row.
                        h = scr.tile([128, P], ACC_DT, name="h")
                        SPLIT = 1792
                        nc.scalar.activation(
                            out=h[:, :SPLIT],
                            in_=pt[:, :SPLIT],
                            func=mybir.ActivationFunctionType.Identity,
                            bias=bias,
                            scale=-2.0,
                        )
                        nc.vector.tensor_scalar(
                            out=h[:, SPLIT:],
                            in0=pt[:, SPLIT:],
                            scalar1=-2.0,
                            scalar2=bias,
                            op0=mybir.AluOpType.mult,
                            op1=mybir.AluOpType.add,
                        )
                        nc.vector.tensor_tensor(
                            out=acc[:],
                            in0=acc[:],
                            in1=h[:],
                            op=mybir.AluOpType.min,
                        )
                nc.sync.dma_start(out.ap(), acc[:])
            else:
                for c in range(P_CHUNKS):
                    acc_c = acc[:, c * 512 : (c + 1) * 512]
                    for t in range(R_TILES):
                        g, o = t // 4, (t % 4) * 128
                        pt = psum.tile([128, 512], F32, name="pt")
                        for k in range(K_TILES):
                            nc.tensor.matmul(
                                pt[:],
                                lhsT=yt_sb[:, g, k, o : o + 128],
                                rhs=xt_sb[:, c, k, :],
                                start=(k == 0),
                                stop=(k == K_TILES - 1),
                            )
                        bias = y2t_sb[:, t : t + 1]
                        if t == 0:
                            nc.scalar.activation(
                                out=acc_c,
                                in_=pt[:],
                                func=mybir.ActivationFunctionType.Identity,
                                bias=bias,
                                scale=-2.0,
                            )
                        else:
                            h = scr.tile([128, 512], F32, name="h")
                            nc.scalar.activation(
                                out=h[:],
                                in_=pt[:],
                                func=mybir.ActivationFunctionType.Identity,
                                bias=bias,
                                scale=-2.0,
                            )
                            nc.vector.tensor_tensor(
                                out=acc_c,
                                in0=acc_c,
                                in1=h[:],
                                op=mybir.AluOpType.min,
                            )
                    # Ship each chunk's result as soon as it is final so the
                    # output DMA overlaps the next chunk's compute.
                    nc.sync.dma_start(out.ap()[:, c * 512 : (c + 1) * 512], acc_c)
    nc.compile()
    return nc


_module_cache: bass.Bass | None = None


def _get_module() -> bass.Bass:
    global _module_cache
    if _module_cache is None:
        _module_cache = _build_module()
    return _module_cache


def _to_partition_major(at: np.ndarray, nchunks: int) -> np.ndarray:
    """[D, W] transposed operand -> [128, nchunks, K_TILES, 512] bf16."""
    w = at.shape[1]
    a4 = at.reshape(K_TILES, 128, nchunks, w // nchunks)
    return np.ascontiguousarray(a4.transpose(1, 2, 0, 3).astype(MM_NP))


def _prepare_inputs(x: np.ndarray, y: np.ndarray):
    """Host-side sharding/layout prep. Returns per-core input maps."""
    xt = _to_partition_major(x.T, P_CHUNKS)
    in_maps = []
    for c in range(NCORES):
        yc = y[c * R_LOC : (c + 1) * R_LOC]
        yct = _to_partition_major(yc.T, R_GROUPS)
        y2 = np.einsum("rd,rd->r", yc, yc, dtype=np.float32)
        if USE_FP8:
            y2 = y2 - Y2_SHIFT
        y2t = np.ascontiguousarray(y2.reshape(R_TILES, 128).T)
        in_maps.append({"xt": xt, "yt": yct, "y2t": y2t})
    return in_maps


def _postprocess(x: np.ndarray, accs: np.ndarray) -> np.ndarray:
    """accs: [NCORES, 128, P] partial mins (missing the x2 term)."""
    m = accs.astype(np.float32).min(axis=(0, 1))  # min over cores and lanes
    if USE_FP8:
        m = m + Y2_SHIFT
    x2 = np.einsum("pd,pd->p", x, x, dtype=np.float32)
    sq_min = np.float32((x2 + m).min())
    return np.sqrt(np.maximum(sq_min, np.float32(0.0)), dtype=np.float32)


def kernel(
    predicted_transaction_company: np.ndarray,
    future_transaction_companies_inc_current_data: np.ndarray,
) -> np.ndarray:
    x = np.asarray(predicted_transaction_company, dtype=np.float32)[0]
    y = np.asarray(future_transaction_companies_inc_current_data, dtype=np.float32)[0]

    nc = _get_module()
    in_maps = _prepare_inputs(x, y)
    res = bass_utils.run_bass_kernel_spmd(nc, in_maps, core_ids=list(range(NCORES)))
    accs = np.stack([r["out"] for r in res.results])
    return _postprocess(x, accs)


# revision 22
# speedup vs baseline: 1.0320x; 1.0320x over previous
"""Min-Euclidean-distance retrieval kernel for Trainium2 (8 NeuronCores).

Reference computation:
    x: [1, 2048, 512], y: [1, 65536, 512] (fp32)
    sq[p, r] = ||x_p||^2 + ||y_r||^2 - 2 <x_p, y_r>
    out = min over (p, r) of sqrt(max(sq, 0))

Sharding: the candidate pool (R) is split across 8 cores, 8192 candidates
each. The host pre-arranges both GEMM operands partition-major in bf16 so
each DMA moves long contiguous per-partition runs and the contraction dim
(d) lands on SBUF partitions with no on-chip transposes.

Per core, per PSUM tile [128 candidates x 512 queries] (bf16 matmuls run
the PE at 1 cycle/row):
  ScalarE:  h = -2*G + y2[r]          (per-partition bias, fp32)
  VectorE:  acc = min(acc, h)         (elementwise across candidate tiles)
The per-query ||x_p||^2 term is constant across candidates, so it is added
on the host, along with the final min across lanes/cores and the
(monotone) sqrt. bf16 operands cost ~1e-4 relative error on the final
distance, far inside tolerance; y2/x2 stay fp32.
"""

import sys

for _p in ("/opt/trn_rl_repo", "/root/.axon_site/_ro/trn_rl_repo"):
    if _p not in sys.path:
        sys.path.append(_p)

import ml_dtypes
import numpy as np

import concourse.bass as bass
import concourse.mybir as mybir
import concourse.tile as tile
from concourse import bacc, bass_utils

P = 2048          # queries
R = 65536         # candidates (full)
D = 512           # feature dim
NCORES = 8
R_LOC = R // NCORES      # 8192 candidates per core
P_CHUNKS = P // 512      # 4 moving chunks of queries
R_TILES = R_LOC // 128   # 64 stationary tiles of candidates
R_GROUPS = 16            # DMA granularity for y: 512 candidates per group
K_TILES = D // 128       # 4 contraction tiles

F32 = mybir.dt.float32
BF16 = mybir.dt.bfloat16
# fp8 e4m3 + DoubleRow runs the PE at 2x bf16 rate; measured rel err on the
# reference inputs is ~1.6e-3 vs 1.1e-4 for bf16. Set USE_FP8=False
# to fall back to the safe bf16 kernel (243 us, 1.1e-4).
USE_FP8 = True
# In fp8 mode the epilogue (ACT combine + DVE min chain) runs in bf16 for
# 2x engine modes. A constant shift keeps the values that matter (near the
# global min, sq ~ 650 => h ~ 150) in a bf16 range with quantum ~1, so the
# extra quantization error (~2e-4 on the distance) is negligible next to
# the fp8 GEMM noise. min-accumulation itself is exact in any format.
Y2_SHIFT = np.float32(512.0)
if USE_FP8:
    MM_DT = mybir.dt.float8e4
    MM_NP = ml_dtypes.float8_e4m3
    ACC_DT, ACC_NP = BF16, ml_dtypes.bfloat16
else:
    MM_DT = mybir.dt.bfloat16
    MM_NP = ml_dtypes.bfloat16
    ACC_DT, ACC_NP = F32, np.float32


def _build_module() -> bass.Bass:
    nc = bacc.Bacc("TRN2", target_bir_lowering=False, debug=False)

    # Host-prepared layouts (partition-major, contiguous per partition):
    #   xt[q, c, k, j] = x[c*512 + j, k*128 + q]
    #   yt[q, g, k, s] = y[g*512 + s, k*128 + q]
    #   y2t[lane, t]   = ||y_r||^2 for r = t*128 + lane
    xt = nc.dram_tensor("xt", [128, P_CHUNKS, K_TILES, 512], MM_DT,
                        kind="ExternalInput")
    yt = nc.dram_tensor("yt", [128, R_GROUPS, K_TILES, 512], MM_DT,
                        kind="ExternalInput")
    y2t = nc.dram_tensor("y2t", [128, R_TILES], F32, kind="ExternalInput")
    # acc[lane, c*512 + j] = min over r-tiles t of
    #   (y2[t*128+lane] - 2 G[t*128+lane, c*512+j])
    out = nc.dram_tensor("out", [128, P], ACC_DT, kind="ExternalOutput")

    with tile.TileContext(nc) as tc:
        with (
            tc.tile_pool(name="big", bufs=1) as big,
            tc.tile_pool(name="scr", bufs=4) as scr,
            tc.tile_pool(name="psum", bufs=(2 if USE_FP8 else 8), space="PSUM") as psum,
        ):
            xt_sb = big.tile([128, P_CHUNKS, K_TILES, 512], MM_DT)
            yt_sb = big.tile([128, R_GROUPS, K_TILES, 512], MM_DT)
            y2t_sb = big.tile([128, R_TILES], F32)
            acc = big.tile([128, P], ACC_DT)

            # First PSUM tile needs x chunk 0 + y group 0 (+ y2t before the
            # first ACT). x goes on the scalar HWDGE ring, y on the sync
            # ring (parallel), y2t via gpsimd SWDGE (its 256B/partition
            # descriptors crawl on the HWDGE ring). Per-group y DMAs let
            # matmuls unblock progressively instead of waiting for one
            # monolithic transfer.
            nc.scalar.dma_start(xt_sb[:, 0], xt.ap()[:, 0])
            nc.sync.dma_start(yt_sb[:, 0], yt.ap()[:, 0])
            nc.gpsimd.dma_start(y2t_sb[:], y2t.ap())
            for g in range(1, R_GROUPS):
                nc.sync.dma_start(yt_sb[:, g], yt.ap()[:, g])
            for c in range(1, P_CHUNKS):
                nc.scalar.dma_start(xt_sb[:, c], xt.ap()[:, c])

            if USE_FP8:
                # DoubleRow: each matmul contracts 2x128 packed K-rows at
                # 2 rows/cycle. One [128, 2048] PSUM tensor (4 banks) holds
                # all query chunks of a candidate tile, so the epilogue is a
                # single wide ACT + DVE op per tile (amortized overheads —
                # the serial DVE min chain is the throughput limit here).
                for t in range(R_TILES):
                    g, o = t // 4, (t % 4) * 128
                    pt = psum.tile([128, P], F32, name="pt")
                    for c in range(P_CHUNKS):
                        for kk in range(K_TILES // 2):
                            nc.tensor.matmul(
                                pt[:, c * 512 : (c + 1) * 512],
                                lhsT=yt_sb[:, g, 2 * kk : 2 * kk + 2, o : o + 128],
                                rhs=xt_sb[:, c, 2 * kk : 2 * kk + 2, :],
                                start=(kk == 0),
                                stop=(kk == K_TILES // 2 - 1),
                                perf_mode=mybir.MatmulPerfMode.DoubleRow,
                            )
                    bias = y2t_sb[:, t : t + 1]
                    if t == 0:
                        nc.scalar.activation(
                            out=acc[:],
                            in_=pt[:],
                            func=mybir.ActivationFunctionType.Identity,
                            bias=bias,
                            scale=-2.0,
                        )
                    else:
                        h = scr.tile([128, P], ACC_DT, name="h")
                        nc.scalar.activation(
                            out=h[:],
                            in_=pt[:],
                            func=mybir.ActivationFunctionType.Identity,
                            bias=bias,
                            scale=-2.0,
                        )
                        nc.vector.tensor_tensor(
                            out=acc[:],
                            in0=acc[:],
                            in1=h[:],
                            op=mybir.AluOpType.min,
                        )
                nc.sync.dma_start(out.ap(), acc[:])
            else:
                for c in range(P_CHUNKS):
                    acc_c = acc[:, c * 512 : (c + 1) * 512]
                    for t in range(R_TILES):
                        g, o = t // 4, (t % 4) * 128
                        pt = psum.tile([128, 512], F32, name="pt")
                        for k in range(K_TILES):
                            nc.tensor.matmul(
                                pt[:],
                                lhsT=yt_sb[:, g, k, o : o + 128],
                                rhs=xt_sb[:, c, k, :],
                                start=(k == 0),
                                stop=(k == K_TILES - 1),
                            )
                        bias = y2t_sb[:, t : t + 1]
                        if t == 0:
                            nc.scalar.activation(
                                out=acc_c,
                                in_=pt[:],
                                func=mybir.ActivationFunctionType.Identity,
                                bias=bias,
                                scale=-2.0,
                            )
                        else:
                            h = scr.tile([128, 512], F32, name="h")
                            nc.scalar.activation(
                                out=h[:],
                                in_=pt[:],
                                func=mybir.ActivationFunctionType.Identity,
                                bias=bias,
                                scale=-2.0,
                            )
                            nc.vector.tensor_tensor(
                                out=acc_c,
                                in0=acc_c,
                                in1=h[:],
                                op=mybir.AluOpType.min,
                            )
                    # Ship each chunk's result as soon as it is final so the
                    # output DMA overlaps the next chunk's compute.
                    nc.sync.dma_start(out.ap()[:, c * 512 : (c + 1) * 512], acc_c)
    nc.compile()
    return nc


_module_cache: bass.Bass | None = None


def _get_module() -> bass.Bass:
    global _module_cache
    if _module_cache is None:
        _module_cache = _build_module()
    return _module_cache


def _to_partition_major(at: np.ndarray, nchunks: int) -> np.ndarray:
    """[D, W] transposed operand -> [128, nchunks, K_TILES, 512] bf16."""
    w = at.shape[1]
    a4 = at.reshape(K_TILES, 128, nchunks, w // nchunks)
    return np.ascontiguousarray(a4.transpose(1, 2, 0, 3).astype(MM_NP))


def _prepare_inputs(x: np.ndarray, y: np.ndarray):
    """Host-side sharding/layout prep. Returns per-core input maps."""
    xt = _to_partition_major(x.T, P_CHUNKS)
    in_maps = []
    for c in range(NCORES):
        yc = y[c * R_LOC : (c + 1) * R_LOC]
        yct = _to_partition_major(yc.T, R_GROUPS)
        y2 = np.einsum("rd,rd->r", yc, yc, dtype=np.float32)
        if USE_FP8:
            y2 = y2 - Y2_SHIFT
        y2t = np.ascontiguousarray(y2.reshape(R_TILES, 128).T)
        in_maps.append({"xt": xt, "yt": yct, "y2t": y2t})
    return in_maps


def _postprocess(x: np.ndarray, accs: np.ndarray) -> np.ndarray:
    """accs: [NCORES, 128, P] partial mins (missing the x2 term)."""
    m = accs.astype(np.float32).min(axis=(0, 1))  # min over cores and lanes
    if USE_FP8:
        m = m + Y2_SHIFT
    x2 = np.einsum("pd,pd->p", x, x, dtype=np.float32)
    sq_min = np.float32((x2 + m).min())
    return np.sqrt(np.maximum(sq_min, np.float32(0.0)), dtype=np.float32)


def kernel(
    predicted_transaction_company: np.ndarray,
    future_transaction_companies_inc_current_data: np.ndarray,
) -> np.ndarray:
    x = np.asarray(predicted_transaction_company, dtype=np.float32)[0]
    y = np.asarray(future_transaction_companies_inc_current_data, dtype=np.float32)[0]

    nc = _get_module()
    in_maps = _prepare_inputs(x, y)
    res = bass_utils.run_bass_kernel_spmd(nc, in_maps, core_ids=list(range(NCORES)))
    accs = np.stack([r["out"] for r in res.results])
    return _postprocess(x, accs)
